# revision 24
# baseline (speedup 1.0000x reference)
"""Trainium2 Bass kernel for BertSelfAttention variant (logsigmoid-fused QK attention).

Reference computation (B=2, S=2048, D=1024, H=16, dh=64):
    q = split_heads(hidden @ Wq + bq)
    k = split_heads(hidden @ Wk + bk)
    k1 = logsigmoid(q) + q + k ; k2 = logsigmoid(k1)
    scores = -(q @ k2^T) / 8 ; probs = softmax(scores) ; ctx = probs @ q

With kk := -k2 = softplus(softplus(-q) - q - k) >= 0, scores == +(q . kk)/8, and
exactly kk = ln(1 + e^{-q-k} + e^{-2q-k}).  e^{-q-k} and e^{-2q-k} come straight
from PSUM via two extra projection chains with host-fused weights (Wq+Wk and
2Wq+Wk), so the whole elementwise chain is: 2 ACT Exps + 1 DVE add per chunk +
one batched ACT Ln per half (exp/ln share one activation table load).

Sharding: 8 cores = 2 (batch) x 4 (head groups of 4 heads / 256 cols of Wq,Wk).
Each core computes its [2048, 256] slice of the output; host reassembles.

Device layout is fully transposed so no matmul ever needs a transposed operand:
    qT, kkT [256(dout), 2048(s)]  from  ht = hidden[b].T  (host-side transpose)
    scoresT[kpos, q] = kkT_head^T @ qT_head    (row-packed head pairs, K=64)
    expT = Exp(scoresT / 8)                    (one [128,1024] ACT op per chunk)
    ctx_aug[65, q] = sum_kpos v_aug[kpos,65]^T @ expT[kpos, q]
        v_aug = [v | 1] -> row 64 accumulates the softmax denominator.
Raw transposed ctx + denominators go back to DRAM; the host divides and
re-transposes while unsharding (no device-side finalize transposes).

Schedule: single software pipeline.  Streams are t-major; while ACT streams the
exps of stream i, the PE drains stream i-1's ctx accumulation (descending kc so
only the first drain matmul carries a semaphore wait), and the projection
matmuls for the second half + the v_aug PE-transposes ride in the leftover PE
slots of the early streams.  One shared 8-bank PSUM pool with tag sharing.

Matmuls run in bf16 (measured end-to-end absmax rel err ~6.4e-3); the softmax
denominator path and output stay fp32.
"""

import numpy as np

B, S, D = 2, 2048, 1024
H, DH = 16, 64
NCORES = 8
HG = 4  # head-group count (tensor parallel)
CPG = (H // HG) * DH  # cols per group = 256
NDT = D // 128  # 8 din tiles
NSC = S // 512  # 4 s-chunks (projection) == 4 q-chunks (attention)
NKC = S // 128  # 16 kpos chunks

MM_DTYPE = "bf16"  # "bf16" | "f32r" | "f32"

_compiled = None
LAST_RESULT = None


def _build():
    from contextlib import ExitStack

    import concourse.bacc as bacc
    import concourse.mybir as mybir
    import concourse.tile as tile

    f32 = mybir.dt.float32
    mmdt = {
        "bf16": mybir.dt.bfloat16,
        "f32r": mybir.dt.float32r,
        "f32": mybir.dt.float32,
    }[MM_DTYPE]
    AF = mybir.ActivationFunctionType

    nc = bacc.Bacc("TRN2", target_bir_lowering=False, debug=False)
    ht = nc.dram_tensor("ht", [D, S], mmdt, kind="ExternalInput").ap()
    wq = nc.dram_tensor("wq", [D, CPG], mmdt, kind="ExternalInput").ap()
    wa = nc.dram_tensor("wa", [D, CPG], mmdt, kind="ExternalInput").ap()
    wb = nc.dram_tensor("wb", [D, CPG], mmdt, kind="ExternalInput").ap()
    # smalls cols: pbq[0:2] nbqk[2:4] nb2qk[4:6] ones[6:22]
    smalls = nc.dram_tensor("smalls", [128, 22], f32, kind="ExternalInput").ap()
    # identity for the v transposes, in the matmul dtype (I64 stacked twice)
    idb = nc.dram_tensor("idb", [128, 64], mmdt, kind="ExternalInput").ap()
    out = nc.dram_tensor("out", [CPG, S], f32, kind="ExternalOutput").ap()
    dens = nc.dram_tensor("dens", [16, S // 4], f32, kind="ExternalOutput").ap()

    with tile.TileContext(nc) as tc, ExitStack() as ctx:
        const = ctx.enter_context(tc.tile_pool(name="const", bufs=1))
        big = ctx.enter_context(tc.tile_pool(name="big", bufs=1))
        sb = ctx.enter_context(tc.tile_pool(name="sb", bufs=3))
        # One PSUM pool for the whole kernel, exactly 8 banks:
        #   qp(x2) ap bp | sp ([128,1024] = 2 banks, x2)
        # ctx tiles share the qp/ap tags (free by the time drains start) and
        # the v_aug transpose tiles share the bp tag.
        ps = ctx.enter_context(tc.tile_pool(name="ps", bufs=1, space="PSUM"))
        etp = ctx.enter_context(tc.tile_pool(name="etp", bufs=20))
        csp = ctx.enter_context(tc.tile_pool(name="csp", bufs=3))

        sm = const.tile([128, 22], f32, tag="smalls")
        nc.sync.dma_start(sm[:], smalls[:])
        pbq_t = sm[:, 0:2]
        nbqk_t = sm[:, 2:4]
        nb2qk_t = sm[:, 4:6]
        ones_t = sm[:, 6:22]
        idb_t = const.tile([128, 64], mmdt, tag="idb")
        nc.sync.dma_start(idb_t[:], idb[:])

        wqs, was, wbs, hts = [], [], [], []
        for j in range(NDT):
            for nm, dram, lst in (("wq", wq, wqs), ("wa", wa, was), ("wb", wb, wbs)):
                w = const.tile([128, CPG], mmdt, tag=f"{nm}{j}", name=f"{nm}s{j}")
                nc.sync.dma_start(w[:], dram[j * 128 : (j + 1) * 128, :])
                lst.append(w)
            t_ = big.tile([128, S], mmdt, tag=f"ht{j}", name=f"hts{j}")
            hts.append(t_)
        # hidden-state chunks sc-major (the first projection chain needs all
        # 8 din tiles of chunk 0 before it can finish), split across both
        # HWDGE rings (SP + ACT) -- the ACT-queue trigger is cheap and the
        # rings run concurrently, halving the DMA-serial lead-in.
        for sc in range(NSC):
            for j in range(NDT):
                eng = nc.sync if (j + sc) % 2 == 0 else nc.scalar
                eng.dma_start(
                    hts[j][:, sc * 512 : (sc + 1) * 512],
                    ht[j * 128 : (j + 1) * 128, sc * 512 : (sc + 1) * 512],
                )

        q_sb = [big.tile([128, S], mmdt, tag=f"q{t}", name=f"q{t}") for t in range(2)]
        kk_sb = [big.tile([128, S], mmdt, tag=f"kk{t}", name=f"kk{t}") for t in range(2)]
        kst = [big.tile([128, S], f32, tag=f"kst{t}", name=f"kst{t}") for t in range(2)]
        vaug = [
            big.tile([128, NKC * 65], mmdt, tag=f"v{h}", name=f"v{h}") for h in range(4)
        ]

        # ---------- emission helpers ----------
        def emit_proj_chunk(t, sc):
            """Three matmul chains for one [dout-half, 512] chunk + elementwise."""
            ssl = slice(sc * 512, (sc + 1) * 512)
            qp = ps.tile([128, 512], f32, tag="qp", name="qp", bufs=2)
            for j in range(NDT):
                nc.tensor.matmul(
                    qp[:],
                    lhsT=wqs[j][:, t * 128 : (t + 1) * 128],
                    rhs=hts[j][:, ssl],
                    start=(j == 0),
                    stop=(j == NDT - 1),
                )
            ap = ps.tile([128, 512], f32, tag="ap", name="ap", bufs=1)
            for j in range(NDT):
                nc.tensor.matmul(
                    ap[:],
                    lhsT=was[j][:, t * 128 : (t + 1) * 128],
                    rhs=hts[j][:, ssl],
                    start=(j == 0),
                    stop=(j == NDT - 1),
                )
            bp = ps.tile([128, 512], f32, tag="bp", name="bp", bufs=1)
            for j in range(NDT):
                nc.tensor.matmul(
                    bp[:],
                    lhsT=wbs[j][:, t * 128 : (t + 1) * 128],
                    rhs=hts[j][:, ssl],
                    start=(j == 0),
                    stop=(j == NDT - 1),
                )
            eu = sb.tile([128, 512], f32, tag="eu")
            nc.scalar.activation(
                eu[:], ap[:], AF.Exp, bias=nbqk_t[:, t : t + 1], scale=-1.0
            )
            ev = sb.tile([128, 512], f32, tag="ev")
            nc.scalar.activation(
                ev[:], bp[:], AF.Exp, bias=nb2qk_t[:, t : t + 1], scale=-1.0
            )
            nc.vector.tensor_add(kst[t][:, ssl], eu[:], ev[:])
            nc.vector.tensor_scalar_add(q_sb[t][:, ssl], qp[:], pbq_t[:, t : t + 1])

        def emit_ln(t):
            nc.scalar.activation(kk_sb[t][:], kst[t][:], AF.Ln, bias=1.0, scale=1.0)

        def emit_vaug_ones(t):
            for rr in range(2):
                vv = vaug[2 * t + rr][:].rearrange("p (c w) -> p c w", w=65)
                nc.vector.tensor_copy(
                    vv[:, :, 64:65], ones_t.rearrange("p (c w) -> p c w", w=1)
                )

        def emit_vaug_chunk(t, j):
            """PE-transpose one [64,128] q chunk per head of half t."""
            for rr in range(2):
                lh = 2 * t + rr
                hsl = slice(rr * 64, rr * 64 + 64)
                tpv = ps.tile([128, 64], mmdt, tag="bp", name="tpv", bufs=1)
                nc.tensor.transpose(
                    tpv[:], q_sb[t][hsl, j * 128 : (j + 1) * 128], idb_t[hsl, 0:64]
                )
                nc.vector.tensor_copy(vaug[lh][:, j * 65 : j * 65 + 64], tpv[:])

        def emit_drain_chunk(prev_state, kc_rev, immediate=False):
            qc_p, t_p, ets_p, ctxs_p = prev_state
            for rr in range(2):
                nc.tensor.matmul(
                    ctxs_p[rr][:],
                    lhsT=vaug[2 * t_p + rr][:, kc_rev * 65 : kc_rev * 65 + 65],
                    rhs=ets_p[kc_rev][:, rr * 512 : rr * 512 + 512],
                    start=(kc_rev == (0 if immediate else NKC - 1)),
                    stop=(kc_rev == (NKC - 1 if immediate else 0)),
                )

        def emit_finalize(prev_state):
            qc_p, t_p, ets_p, ctxs_p = prev_state
            qsl_p = slice(qc_p * 512, (qc_p + 1) * 512)
            for rr in range(2):
                lh = 2 * t_p + rr
                cs = csp.tile([128, 512], f32, tag="cs")
                nc.vector.tensor_copy(cs[0:65, :], ctxs_p[rr][:])
                nc.sync.dma_start(out[lh * 64 : lh * 64 + 64, qsl_p], cs[0:64, :])
                nc.sync.dma_start(
                    dens[qc_p * 4 + t_p * 2 + rr : qc_p * 4 + t_p * 2 + rr + 1, :],
                    cs[64:65, :],
                )

        def run_filler(item):
            if item[0] == "vaug":
                emit_vaug_chunk(item[1], item[2])
            elif item[0] == "proj":
                emit_proj_chunk(item[1], item[2])
            elif item[0] == "ln":
                emit_ln(item[1])

        # ---------- schedule ----------
        # First-half projection, then one long pipeline of 8 t-major streams.
        for sc in range(NSC):
            emit_proj_chunk(0, sc)
        emit_ln(0)
        emit_vaug_ones(0)
        emit_vaug_ones(1)

        # extra PE work interleaved into the streams' spare PE slots
        filler = {
            0: [("vaug", 0, j) for j in range(NKC)],
            1: [("proj", 1, 0), ("proj", 1, 1)],
            2: [("proj", 1, 2), ("proj", 1, 3), ("ln", 1)],
            4: [("vaug", 1, j) for j in range(NKC)],
        }

        streams = [(qc, t) for t in range(2) for qc in range(NSC)]
        prev = None
        for i, (qc, t) in enumerate(streams):
            qsl = slice(qc * 512, (qc + 1) * 512)
            fill = list(filler.get(i, []))
            last = i == len(streams) - 1
            ets = []
            ctxs_now = None
            if last:
                # final stream: drain immediately per chunk (ascending kc), so
                # only the finalize remains after the pipeline.
                ctxs_now = [
                    ps.tile([65, 512], f32, tag="qp", name="ctxA", bufs=2),
                    ps.tile([65, 512], f32, tag="ap", name="ctxB", bufs=1),
                ]
            for kc in range(NKC):
                ksl = slice(kc * 128, (kc + 1) * 128)
                sp = ps.tile([128, 1024], f32, tag="sp", name="sp", bufs=2)
                nc.tensor.matmul(
                    sp[:, 0:512],
                    lhsT=kk_sb[t][0:64, ksl],
                    rhs=q_sb[t][0:64, qsl],
                    start=True,
                    stop=True,
                )
                nc.tensor.matmul(
                    sp[:, 512:1024],
                    lhsT=kk_sb[t][64:128, ksl],
                    rhs=q_sb[t][64:128, qsl],
                    start=True,
                    stop=True,
                )
                et = etp.tile([128, 1024], mmdt, tag="et", name=f"et{kc}")
                nc.scalar.activation(et[:], sp[:], AF.Exp, scale=0.125)
                ets.append(et)
                if prev is not None:
                    emit_drain_chunk(prev, NKC - 1 - kc)
                if last:
                    emit_drain_chunk((qc, t, ets, ctxs_now), kc, immediate=True)
                # interleave one filler item per chunk slot (back-loaded so the
                # filler's dependencies have time to resolve)
                if fill and (kc % 2 == 1 or len(fill) >= NKC - kc):
                    run_filler(fill.pop(0))
            for item in fill:
                run_filler(item)
            if prev is not None:
                emit_finalize(prev)
            if last:
                emit_finalize((qc, t, ets, ctxs_now))
                prev = None
            else:
                ctxs = [
                    ps.tile([65, 512], f32, tag="qp", name="ctxA", bufs=2),
                    ps.tile([65, 512], f32, tag="ap", name="ctxB", bufs=1),
                ]
                prev = (qc, t, ets, ctxs)

    nc.compile()
    return nc


def kernel(hidden_states, attention_mask, Wq, bq, Wk, bk):
    global _compiled, LAST_RESULT
    hs = np.asarray(hidden_states, dtype=np.float32)
    am = np.asarray(attention_mask)
    Wq = np.asarray(Wq, dtype=np.float32)
    Wk = np.asarray(Wk, dtype=np.float32)
    bq = np.asarray(bq, dtype=np.float32)
    bk = np.asarray(bk, dtype=np.float32)

    if _compiled is None:
        _compiled = _build()
    nc = _compiled

    from concourse.bass_utils import run_bass_kernel_spmd

    if MM_DTYPE == "bf16":
        import ml_dtypes

        def to_mmdt(x):
            return np.ascontiguousarray(
                np.asarray(x, np.float32).astype(ml_dtypes.bfloat16)
            )

    elif MM_DTYPE == "f32r":

        def to_mmdt(x):
            # fp32r = 1s/8e/11m (top 20 bits of fp32), round-to-nearest-even
            b = np.ascontiguousarray(x, dtype=np.float32).view(np.uint32)
            lsb = (b >> np.uint32(12)) & np.uint32(1)
            r = (b + np.uint32(0x7FF) + lsb) & np.uint32(0xFFFFF000)
            return r.view(np.float32)

    else:

        def to_mmdt(x):
            return np.ascontiguousarray(x, dtype=np.float32)

    idb = to_mmdt(np.tile(np.eye(64, dtype=np.float32), (2, 1)))
    in_maps = []
    for c in range(NCORES):
        b, g = c // HG, c % HG
        cols = slice(g * CPG, (g + 1) * CPG)
        bq_s = bq[cols].reshape(2, 128).T
        bk_s = bk[cols].reshape(2, 128).T
        smalls = np.concatenate(
            [bq_s, -(bq_s + bk_s), -(2 * bq_s + bk_s), np.ones((128, 16), np.float32)],
            axis=1,
        ).astype(np.float32)
        in_maps.append(
            {
                "ht": to_mmdt(hs[b].T),
                "wq": to_mmdt(Wq[:, cols]),
                "wa": to_mmdt(Wq[:, cols] + Wk[:, cols]),
                "wb": to_mmdt(2.0 * Wq[:, cols] + Wk[:, cols]),
                "smalls": np.ascontiguousarray(smalls),
                "idb": idb,
            }
        )

    res = run_bass_kernel_spmd(nc, in_maps, list(range(NCORES)))
    LAST_RESULT = res

    outp = np.empty((B, S, H * DH), dtype=np.float32)
    for c in range(NCORES):
        b, g = c // HG, c % HG
        ctxT = res.results[c]["out"]  # [256, 2048] raw ctx sums (transposed)
        dn = res.results[c]["dens"]  # [16, 512]: row qc*4 + t*2 + rr
        den = np.empty((4, S), dtype=np.float32)
        for qc in range(NSC):
            for t in range(2):
                for rr in range(2):
                    den[t * 2 + rr, qc * 512 : (qc + 1) * 512] = dn[qc * 4 + t * 2 + rr]
        ctxT = ctxT.reshape(4, 64, S) / den[:, None, :]
        outp[b, :, g * CPG : (g + 1) * CPG] = ctxT.reshape(CPG, S).T

    # attention_mask==0 masks whole query rows -> uniform probs -> ctx row is
    # the mean of q over all key positions. Never triggers for all-ones masks.
    if (am == 0).any():
        for b in range(B):
            rows = np.nonzero(am[b] == 0)[0]
            if rows.size:
                q_full = hs[b] @ Wq + bq
                outp[b, rows, :] = q_full.mean(axis=0)
    return outp


# revision 25
# speedup vs baseline: 1.0263x; 1.0263x over previous
"""Trainium2 Bass kernel for BertSelfAttention variant (logsigmoid-fused QK attention).

Reference computation (B=2, S=2048, D=1024, H=16, dh=64):
    q = split_heads(hidden @ Wq + bq)
    k = split_heads(hidden @ Wk + bk)
    k1 = logsigmoid(q) + q + k ; k2 = logsigmoid(k1)
    scores = -(q @ k2^T) / 8 ; probs = softmax(scores) ; ctx = probs @ q

With kk := -k2 = softplus(softplus(-q) - q - k) >= 0, scores == +(q . kk)/8, and
exactly kk = ln(1 + e^{-q-k} + e^{-2q-k}).  e^{-q-k} and e^{-2q-k} come straight
from PSUM via two extra projection chains with host-fused weights (Wq+Wk and
2Wq+Wk), so the whole elementwise chain is: 2 ACT Exps + 1 DVE add per chunk +
one batched ACT Ln per half (exp/ln share one activation table load).

Sharding: 8 cores = 2 (batch) x 4 (head groups of 4 heads / 256 cols of Wq,Wk).
Each core computes its [2048, 256] slice of the output; host reassembles.

Device layout is fully transposed so no matmul ever needs a transposed operand:
    qT, kkT [256(dout), 2048(s)]  from  ht = hidden[b].T  (host-side transpose)
    scoresT[kpos, q] = kkT_head^T @ qT_head    (row-packed head pairs, K=64)
    expT = Exp(scoresT / 8)                    (one [128,1024] ACT op per chunk)
    ctx_aug[65, q] = sum_kpos v_aug[kpos,65]^T @ expT[kpos, q]
        v_aug = [v | 1] -> row 64 accumulates the softmax denominator.
Raw transposed ctx + denominators go back to DRAM; the host divides and
re-transposes while unsharding (no device-side finalize transposes).

Schedule: single software pipeline.  Streams are t-major; while ACT streams the
exps of stream i, the PE drains stream i-1's ctx accumulation (descending kc so
only the first drain matmul carries a semaphore wait), and the projection
matmuls for the second half + the v_aug PE-transposes ride in the leftover PE
slots of the early streams.  One shared 8-bank PSUM pool with tag sharing.

Matmuls run in bf16 (measured end-to-end absmax rel err ~6.4e-3); the softmax
denominator path and output stay fp32.
"""

import numpy as np

B, S, D = 2, 2048, 1024
H, DH = 16, 64
NCORES = 8
HG = 4  # head-group count (tensor parallel)
CPG = (H // HG) * DH  # cols per group = 256
NDT = D // 128  # 8 din tiles
NSC = S // 512  # 4 s-chunks (projection) == 4 q-chunks (attention)
NKC = S // 128  # 16 kpos chunks

MM_DTYPE = "bf16"  # "bf16" | "f32r" | "f32"

_compiled = None
LAST_RESULT = None


def _build():
    from contextlib import ExitStack

    import concourse.bacc as bacc
    import concourse.mybir as mybir
    import concourse.tile as tile

    f32 = mybir.dt.float32
    mmdt = {
        "bf16": mybir.dt.bfloat16,
        "f32r": mybir.dt.float32r,
        "f32": mybir.dt.float32,
    }[MM_DTYPE]
    AF = mybir.ActivationFunctionType

    nc = bacc.Bacc("TRN2", target_bir_lowering=False, debug=False)
    ht = nc.dram_tensor("ht", [D, S], mmdt, kind="ExternalInput").ap()
    wq = nc.dram_tensor("wq", [D, CPG], mmdt, kind="ExternalInput").ap()
    wa = nc.dram_tensor("wa", [D, CPG], mmdt, kind="ExternalInput").ap()
    wb = nc.dram_tensor("wb", [D, CPG], mmdt, kind="ExternalInput").ap()
    # smalls cols: pbq[0:2] nbqk[2:4] nb2qk[4:6] ones[6:22]
    smalls = nc.dram_tensor("smalls", [128, 22], f32, kind="ExternalInput").ap()
    # identity for the v transposes, in the matmul dtype (I64 stacked twice)
    idb = nc.dram_tensor("idb", [128, 64], mmdt, kind="ExternalInput").ap()
    out = nc.dram_tensor("out", [CPG, S], f32, kind="ExternalOutput").ap()
    dens = nc.dram_tensor("dens", [16, S // 4], f32, kind="ExternalOutput").ap()

    with tile.TileContext(nc) as tc, ExitStack() as ctx:
        const = ctx.enter_context(tc.tile_pool(name="const", bufs=1))
        big = ctx.enter_context(tc.tile_pool(name="big", bufs=1))
        sb = ctx.enter_context(tc.tile_pool(name="sb", bufs=3))
        # One PSUM pool for the whole kernel, exactly 8 banks:
        #   qp(x2) ap bp | sp ([128,1024] = 2 banks, x2)
        # ctx tiles share the qp/ap tags (free by the time drains start) and
        # the v_aug transpose tiles share the bp tag.
        ps = ctx.enter_context(tc.tile_pool(name="ps", bufs=1, space="PSUM"))
        etp = ctx.enter_context(tc.tile_pool(name="etp", bufs=20))
        csp = ctx.enter_context(tc.tile_pool(name="csp", bufs=3))

        sm = const.tile([128, 22], f32, tag="smalls")
        nc.sync.dma_start(sm[:], smalls[:])
        pbq_t = sm[:, 0:2]
        nbqk_t = sm[:, 2:4]
        nb2qk_t = sm[:, 4:6]
        ones_t = sm[:, 6:22]
        idb_t = const.tile([128, 64], mmdt, tag="idb")
        nc.sync.dma_start(idb_t[:], idb[:])

        # Inputs needed first come first, interleaved across both HWDGE
        # rings (SP + ACT): per din tile j, its three weight chunks plus the
        # sc=0 hidden chunk, so the first projection chain can finish as soon
        # as ~1/4 of the input stream has landed.  Remaining hidden chunks
        # follow sc-major.
        rr_ring = [nc.sync, nc.scalar]
        ring_i = 0

        def ring():
            nonlocal ring_i
            ring_i += 1
            return rr_ring[ring_i % 2]

        wqs, was, wbs, hts = [], [], [], []
        for j in range(NDT):
            t_ = big.tile([128, S], mmdt, tag=f"ht{j}", name=f"hts{j}")
            hts.append(t_)
        for j in range(NDT):
            for nm, dram, lst in (("wq", wq, wqs), ("wa", wa, was), ("wb", wb, wbs)):
                w = const.tile([128, CPG], mmdt, tag=f"{nm}{j}", name=f"{nm}s{j}")
                ring().dma_start(w[:], dram[j * 128 : (j + 1) * 128, :])
                lst.append(w)
            ring().dma_start(
                hts[j][:, 0:512], ht[j * 128 : (j + 1) * 128, 0:512]
            )
        for sc in range(1, NSC):
            for j in range(NDT):
                ring().dma_start(
                    hts[j][:, sc * 512 : (sc + 1) * 512],
                    ht[j * 128 : (j + 1) * 128, sc * 512 : (sc + 1) * 512],
                )

        q_sb = [big.tile([128, S], mmdt, tag=f"q{t}", name=f"q{t}") for t in range(2)]
        kk_sb = [big.tile([128, S], mmdt, tag=f"kk{t}", name=f"kk{t}") for t in range(2)]
        kst = [big.tile([128, S], f32, tag=f"kst{t}", name=f"kst{t}") for t in range(2)]
        vaug = [
            big.tile([128, NKC * 65], mmdt, tag=f"v{h}", name=f"v{h}") for h in range(4)
        ]

        # ---------- emission helpers ----------
        def emit_proj_chunk(t, sc):
            """Three matmul chains for one [dout-half, 512] chunk + elementwise."""
            ssl = slice(sc * 512, (sc + 1) * 512)
            qp = ps.tile([128, 512], f32, tag="qp", name="qp", bufs=2)
            for j in range(NDT):
                nc.tensor.matmul(
                    qp[:],
                    lhsT=wqs[j][:, t * 128 : (t + 1) * 128],
                    rhs=hts[j][:, ssl],
                    start=(j == 0),
                    stop=(j == NDT - 1),
                )
            ap = ps.tile([128, 512], f32, tag="ap", name="ap", bufs=1)
            for j in range(NDT):
                nc.tensor.matmul(
                    ap[:],
                    lhsT=was[j][:, t * 128 : (t + 1) * 128],
                    rhs=hts[j][:, ssl],
                    start=(j == 0),
                    stop=(j == NDT - 1),
                )
            bp = ps.tile([128, 512], f32, tag="bp", name="bp", bufs=1)
            for j in range(NDT):
                nc.tensor.matmul(
                    bp[:],
                    lhsT=wbs[j][:, t * 128 : (t + 1) * 128],
                    rhs=hts[j][:, ssl],
                    start=(j == 0),
                    stop=(j == NDT - 1),
                )
            eu = sb.tile([128, 512], f32, tag="eu")
            nc.scalar.activation(
                eu[:], ap[:], AF.Exp, bias=nbqk_t[:, t : t + 1], scale=-1.0
            )
            ev = sb.tile([128, 512], f32, tag="ev")
            nc.scalar.activation(
                ev[:], bp[:], AF.Exp, bias=nb2qk_t[:, t : t + 1], scale=-1.0
            )
            nc.vector.tensor_add(kst[t][:, ssl], eu[:], ev[:])
            nc.vector.tensor_scalar_add(q_sb[t][:, ssl], qp[:], pbq_t[:, t : t + 1])

        def emit_ln(t):
            nc.scalar.activation(kk_sb[t][:], kst[t][:], AF.Ln, bias=1.0, scale=1.0)

        def emit_vaug_ones(t):
            for rr in range(2):
                vv = vaug[2 * t + rr][:].rearrange("p (c w) -> p c w", w=65)
                nc.vector.tensor_copy(
                    vv[:, :, 64:65], ones_t.rearrange("p (c w) -> p c w", w=1)
                )

        def emit_vaug_chunk(t, j):
            """PE-transpose one [64,128] q chunk per head of half t."""
            for rr in range(2):
                lh = 2 * t + rr
                hsl = slice(rr * 64, rr * 64 + 64)
                tpv = ps.tile([128, 64], mmdt, tag="bp", name="tpv", bufs=1)
                nc.tensor.transpose(
                    tpv[:], q_sb[t][hsl, j * 128 : (j + 1) * 128], idb_t[hsl, 0:64]
                )
                nc.vector.tensor_copy(vaug[lh][:, j * 65 : j * 65 + 64], tpv[:])

        def emit_drain_chunk(prev_state, kc_rev, immediate=False):
            qc_p, t_p, ets_p, ctxs_p = prev_state
            for rr in range(2):
                nc.tensor.matmul(
                    ctxs_p[rr][:],
                    lhsT=vaug[2 * t_p + rr][:, kc_rev * 65 : kc_rev * 65 + 65],
                    rhs=ets_p[kc_rev][:, rr * 512 : rr * 512 + 512],
                    start=(kc_rev == (0 if immediate else NKC - 1)),
                    stop=(kc_rev == (NKC - 1 if immediate else 0)),
                )

        def emit_finalize(prev_state):
            qc_p, t_p, ets_p, ctxs_p = prev_state
            qsl_p = slice(qc_p * 512, (qc_p + 1) * 512)
            for rr in range(2):
                lh = 2 * t_p + rr
                cs = csp.tile([128, 512], f32, tag="cs")
                nc.vector.tensor_copy(cs[0:65, :], ctxs_p[rr][:])
                nc.sync.dma_start(out[lh * 64 : lh * 64 + 64, qsl_p], cs[0:64, :])
                nc.sync.dma_start(
                    dens[qc_p * 4 + t_p * 2 + rr : qc_p * 4 + t_p * 2 + rr + 1, :],
                    cs[64:65, :],
                )

        def run_filler(item):
            if item[0] == "vaug":
                emit_vaug_chunk(item[1], item[2])
            elif item[0] == "proj":
                emit_proj_chunk(item[1], item[2])
            elif item[0] == "ln":
                emit_ln(item[1])

        # ---------- schedule ----------
        # First-half projection, then one long pipeline of 8 t-major streams.
        for sc in range(NSC):
            emit_proj_chunk(0, sc)
        emit_ln(0)
        emit_vaug_ones(0)
        emit_vaug_ones(1)

        # extra PE work interleaved into the streams' spare PE slots
        filler = {
            0: [("vaug", 0, j) for j in range(NKC)],
            1: [("proj", 1, 0), ("proj", 1, 1)],
            2: [("proj", 1, 2), ("proj", 1, 3), ("ln", 1)],
            4: [("vaug", 1, j) for j in range(NKC)],
        }

        streams = [(qc, t) for t in range(2) for qc in range(NSC)]
        prev = None
        for i, (qc, t) in enumerate(streams):
            qsl = slice(qc * 512, (qc + 1) * 512)
            fill = list(filler.get(i, []))
            last = i == len(streams) - 1
            ets = []
            ctxs_now = None
            if last:
                # final stream: drain immediately per chunk (ascending kc), so
                # only the finalize remains after the pipeline.
                ctxs_now = [
                    ps.tile([65, 512], f32, tag="qp", name="ctxA", bufs=2),
                    ps.tile([65, 512], f32, tag="ap", name="ctxB", bufs=1),
                ]
            for kc in range(NKC):
                ksl = slice(kc * 128, (kc + 1) * 128)
                sp = ps.tile([128, 1024], f32, tag="sp", name="sp", bufs=2)
                nc.tensor.matmul(
                    sp[:, 0:512],
                    lhsT=kk_sb[t][0:64, ksl],
                    rhs=q_sb[t][0:64, qsl],
                    start=True,
                    stop=True,
                )
                nc.tensor.matmul(
                    sp[:, 512:1024],
                    lhsT=kk_sb[t][64:128, ksl],
                    rhs=q_sb[t][64:128, qsl],
                    start=True,
                    stop=True,
                )
                et = etp.tile([128, 1024], mmdt, tag="et", name=f"et{kc}")
                nc.scalar.activation(et[:], sp[:], AF.Exp, scale=0.125)
                ets.append(et)
                if prev is not None:
                    emit_drain_chunk(prev, NKC - 1 - kc)
                if last:
                    emit_drain_chunk((qc, t, ets, ctxs_now), kc, immediate=True)
                # interleave one filler item per chunk slot (back-loaded so the
                # filler's dependencies have time to resolve)
                if fill and (kc % 2 == 1 or len(fill) >= NKC - kc):
                    run_filler(fill.pop(0))
            for item in fill:
                run_filler(item)
            if prev is not None:
                emit_finalize(prev)
            if last:
                emit_finalize((qc, t, ets, ctxs_now))
                prev = None
            else:
                ctxs = [
                    ps.tile([65, 512], f32, tag="qp", name="ctxA", bufs=2),
                    ps.tile([65, 512], f32, tag="ap", name="ctxB", bufs=1),
                ]
                prev = (qc, t, ets, ctxs)

    nc.compile()
    return nc


def kernel(hidden_states, attention_mask, Wq, bq, Wk, bk):
    global _compiled, LAST_RESULT
    hs = np.asarray(hidden_states, dtype=np.float32)
    am = np.asarray(attention_mask)
    Wq = np.asarray(Wq, dtype=np.float32)
    Wk = np.asarray(Wk, dtype=np.float32)
    bq = np.asarray(bq, dtype=np.float32)
    bk = np.asarray(bk, dtype=np.float32)

    if _compiled is None:
        _compiled = _build()
    nc = _compiled

    from concourse.bass_utils import run_bass_kernel_spmd

    if MM_DTYPE == "bf16":
        import ml_dtypes

        def to_mmdt(x):
            return np.ascontiguousarray(
                np.asarray(x, np.float32).astype(ml_dtypes.bfloat16)
            )

    elif MM_DTYPE == "f32r":

        def to_mmdt(x):
            # fp32r = 1s/8e/11m (top 20 bits of fp32), round-to-nearest-even
            b = np.ascontiguousarray(x, dtype=np.float32).view(np.uint32)
            lsb = (b >> np.uint32(12)) & np.uint32(1)
            r = (b + np.uint32(0x7FF) + lsb) & np.uint32(0xFFFFF000)
            return r.view(np.float32)

    else:

        def to_mmdt(x):
            return np.ascontiguousarray(x, dtype=np.float32)

    idb = to_mmdt(np.tile(np.eye(64, dtype=np.float32), (2, 1)))
    in_maps = []
    for c in range(NCORES):
        b, g = c // HG, c % HG
        cols = slice(g * CPG, (g + 1) * CPG)
        bq_s = bq[cols].reshape(2, 128).T
        bk_s = bk[cols].reshape(2, 128).T
        smalls = np.concatenate(
            [bq_s, -(bq_s + bk_s), -(2 * bq_s + bk_s), np.ones((128, 16), np.float32)],
            axis=1,
        ).astype(np.float32)
        in_maps.append(
            {
                "ht": to_mmdt(hs[b].T),
                "wq": to_mmdt(Wq[:, cols]),
                "wa": to_mmdt(Wq[:, cols] + Wk[:, cols]),
                "wb": to_mmdt(2.0 * Wq[:, cols] + Wk[:, cols]),
                "smalls": np.ascontiguousarray(smalls),
                "idb": idb,
            }
        )

    res = run_bass_kernel_spmd(nc, in_maps, list(range(NCORES)))
    LAST_RESULT = res

    outp = np.empty((B, S, H * DH), dtype=np.float32)
    for c in range(NCORES):
        b, g = c // HG, c % HG
        ctxT = res.results[c]["out"]  # [256, 2048] raw ctx sums (transposed)
        dn = res.results[c]["dens"]  # [16, 512]: row qc*4 + t*2 + rr
        den = np.empty((4, S), dtype=np.float32)
        for qc in range(NSC):
            for t in range(2):
                for rr in range(2):
                    den[t * 2 + rr, qc * 512 : (qc + 1) * 512] = dn[qc * 4 + t * 2 + rr]
        ctxT = ctxT.reshape(4, 64, S) / den[:, None, :]
        outp[b, :, g * CPG : (g + 1) * CPG] = ctxT.reshape(CPG, S).T

    # attention_mask==0 masks whole query rows -> uniform probs -> ctx row is
    # the mean of q over all key positions. Never triggers for all-ones masks.
    if (am == 0).any():
        for b in range(B):
            rows = np.nonzero(am[b] == 0)[0]
            if rows.size:
                q_full = hs[b] @ Wq + bq
                outp[b, rows, :] = q_full.mean(axis=0)
    return outp


# revision 26
# speedup vs baseline: 1.0545x; 1.0275x over previous
"""Trainium2 Bass kernel for BertSelfAttention variant (logsigmoid-fused QK attention).

Reference computation (B=2, S=2048, D=1024, H=16, dh=64):
    q = split_heads(hidden @ Wq + bq)
    k = split_heads(hidden @ Wk + bk)
    k1 = logsigmoid(q) + q + k ; k2 = logsigmoid(k1)
    scores = -(q @ k2^T) / 8 ; probs = softmax(scores) ; ctx = probs @ q

With kk := -k2 = softplus(softplus(-q) - q - k) >= 0, scores == +(q . kk)/8, and
exactly kk = ln(1 + e^{-q-k} + e^{-2q-k}).  e^{-q-k} and e^{-2q-k} come straight
from PSUM via two extra projection chains with host-fused weights (Wq+Wk and
2Wq+Wk), so the whole elementwise chain is: 2 ACT Exps + 1 DVE add per chunk +
one batched ACT Ln per half (exp/ln share one activation table load).

Sharding: 8 cores = 2 (batch) x 4 (head groups of 4 heads / 256 cols of Wq,Wk).
Each core computes its [2048, 256] slice of the output; host reassembles.

Device layout is fully transposed so no matmul ever needs a transposed operand:
    qT, kkT [256(dout), 2048(s)]  from  ht = hidden[b].T  (host-side transpose)
    scoresT[kpos, q] = kkT_head^T @ qT_head    (row-packed head pairs, K=64)
    expT = Exp(scoresT / 8)                    (one [128,1024] ACT op per chunk)
    ctx_aug[65, q] = sum_kpos v_aug[kpos,65]^T @ expT[kpos, q]
        v_aug = [v | 1] -> row 64 accumulates the softmax denominator.
Raw transposed ctx + denominators go back to DRAM; the host divides and
re-transposes while unsharding (no device-side finalize transposes).

Schedule: single software pipeline.  Streams are t-major; while ACT streams the
exps of stream i, the PE drains stream i-1's ctx accumulation (descending kc so
only the first drain matmul carries a semaphore wait), and the projection
matmuls for the second half + the v_aug PE-transposes ride in the leftover PE
slots of the early streams.  One shared 8-bank PSUM pool with tag sharing.

Matmuls run in bf16 (measured end-to-end absmax rel err ~6.4e-3); the softmax
denominator path and output stay fp32.
"""

import numpy as np

B, S, D = 2, 2048, 1024
H, DH = 16, 64
NCORES = 8
HG = 4  # head-group count (tensor parallel)
CPG = (H // HG) * DH  # cols per group = 256
NDT = D // 128  # 8 din tiles
NSC = S // 512  # 4 s-chunks (projection) == 4 q-chunks (attention)
NKC = S // 128  # 16 kpos chunks

MM_DTYPE = "bf16"  # "bf16" | "f32r" | "f32"

_compiled = None
LAST_RESULT = None


def _build():
    from contextlib import ExitStack

    import concourse.bacc as bacc
    import concourse.mybir as mybir
    import concourse.tile as tile

    f32 = mybir.dt.float32
    mmdt = {
        "bf16": mybir.dt.bfloat16,
        "f32r": mybir.dt.float32r,
        "f32": mybir.dt.float32,
    }[MM_DTYPE]
    AF = mybir.ActivationFunctionType

    nc = bacc.Bacc("TRN2", target_bir_lowering=False, debug=False)
    ht = nc.dram_tensor("ht", [D, S], mmdt, kind="ExternalInput").ap()
    wq = nc.dram_tensor("wq", [D, CPG], mmdt, kind="ExternalInput").ap()
    wa = nc.dram_tensor("wa", [D, CPG], mmdt, kind="ExternalInput").ap()
    wb = nc.dram_tensor("wb", [D, CPG], mmdt, kind="ExternalInput").ap()
    # smalls cols: pbq[0:2] nbqk[2:4] nb2qk[4:6] ones[6:22]
    smalls = nc.dram_tensor("smalls", [128, 22], f32, kind="ExternalInput").ap()
    # identity for the v transposes, in the matmul dtype (I64 stacked twice)
    idb = nc.dram_tensor("idb", [128, 64], mmdt, kind="ExternalInput").ap()
    out = nc.dram_tensor("out", [CPG, S], f32, kind="ExternalOutput").ap()
    dens = nc.dram_tensor("dens", [16, S // 4], f32, kind="ExternalOutput").ap()

    with tile.TileContext(nc) as tc, ExitStack() as ctx:
        const = ctx.enter_context(tc.tile_pool(name="const", bufs=1))
        big = ctx.enter_context(tc.tile_pool(name="big", bufs=1))
        sb = ctx.enter_context(tc.tile_pool(name="sb", bufs=3))
        # One PSUM pool for the whole kernel, exactly 8 banks:
        #   qp(x2) ap bp | sp ([128,1024] = 2 banks, x2)
        # ctx tiles share the qp/ap tags (free by the time drains start) and
        # the v_aug transpose tiles share the bp tag.
        ps = ctx.enter_context(tc.tile_pool(name="ps", bufs=1, space="PSUM"))
        etp = ctx.enter_context(tc.tile_pool(name="etp", bufs=20))
        csp = ctx.enter_context(tc.tile_pool(name="csp", bufs=3))

        sm = const.tile([128, 22], f32, tag="smalls")
        nc.sync.dma_start(sm[:], smalls[:])
        pbq_t = sm[:, 0:2]
        nbqk_t = sm[:, 2:4]
        nb2qk_t = sm[:, 4:6]
        ones_t = sm[:, 6:22]
        idb_t = const.tile([128, 64], mmdt, tag="idb")
        nc.sync.dma_start(idb_t[:], idb[:])

        # Inputs needed first come first, interleaved across both HWDGE
        # rings (SP + ACT): per din tile j, its three weight chunks plus the
        # sc=0 hidden chunk, so the first projection chain can finish as soon
        # as ~1/4 of the input stream has landed.  Remaining hidden chunks
        # follow sc-major.
        rr_ring = [nc.sync, nc.scalar]
        ring_i = 0

        def ring():
            nonlocal ring_i
            ring_i += 1
            return rr_ring[ring_i % 2]

        wqs, was, wbs, hts = [], [], [], []
        for j in range(NDT):
            t_ = big.tile([128, S], mmdt, tag=f"ht{j}", name=f"hts{j}")
            hts.append(t_)
        for j in range(NDT):
            for nm, dram, lst in (("wq", wq, wqs), ("wa", wa, was), ("wb", wb, wbs)):
                w = const.tile([128, CPG], mmdt, tag=f"{nm}{j}", name=f"{nm}s{j}")
                ring().dma_start(w[:], dram[j * 128 : (j + 1) * 128, :])
                lst.append(w)
            ring().dma_start(
                hts[j][:, 0:512], ht[j * 128 : (j + 1) * 128, 0:512]
            )
        # later chunks are not latency-critical: keep them off the ACT ring
        # so its queue is free for the activation compute that starts early.
        for sc in range(1, NSC):
            for j in range(NDT):
                nc.sync.dma_start(
                    hts[j][:, sc * 512 : (sc + 1) * 512],
                    ht[j * 128 : (j + 1) * 128, sc * 512 : (sc + 1) * 512],
                )

        q_sb = [big.tile([128, S], mmdt, tag=f"q{t}", name=f"q{t}") for t in range(2)]
        kk_sb = [big.tile([128, S], mmdt, tag=f"kk{t}", name=f"kk{t}") for t in range(2)]
        kst = [big.tile([128, S], f32, tag=f"kst{t}", name=f"kst{t}") for t in range(2)]
        vaug = [
            big.tile([128, NKC * 65], mmdt, tag=f"v{h}", name=f"v{h}") for h in range(4)
        ]

        # ---------- emission helpers ----------
        def emit_proj_chunk(t, sc):
            """Three matmul chains for one [dout-half, 512] chunk + elementwise."""
            ssl = slice(sc * 512, (sc + 1) * 512)
            qp = ps.tile([128, 512], f32, tag="qp", name="qp", bufs=2)
            for j in range(NDT):
                nc.tensor.matmul(
                    qp[:],
                    lhsT=wqs[j][:, t * 128 : (t + 1) * 128],
                    rhs=hts[j][:, ssl],
                    start=(j == 0),
                    stop=(j == NDT - 1),
                )
            ap = ps.tile([128, 512], f32, tag="ap", name="ap", bufs=1)
            for j in range(NDT):
                nc.tensor.matmul(
                    ap[:],
                    lhsT=was[j][:, t * 128 : (t + 1) * 128],
                    rhs=hts[j][:, ssl],
                    start=(j == 0),
                    stop=(j == NDT - 1),
                )
            bp = ps.tile([128, 512], f32, tag="bp", name="bp", bufs=1)
            for j in range(NDT):
                nc.tensor.matmul(
                    bp[:],
                    lhsT=wbs[j][:, t * 128 : (t + 1) * 128],
                    rhs=hts[j][:, ssl],
                    start=(j == 0),
                    stop=(j == NDT - 1),
                )
            eu = sb.tile([128, 512], f32, tag="eu")
            nc.scalar.activation(
                eu[:], ap[:], AF.Exp, bias=nbqk_t[:, t : t + 1], scale=-1.0
            )
            ev = sb.tile([128, 512], f32, tag="ev")
            nc.scalar.activation(
                ev[:], bp[:], AF.Exp, bias=nb2qk_t[:, t : t + 1], scale=-1.0
            )
            nc.vector.tensor_add(kst[t][:, ssl], eu[:], ev[:])
            nc.vector.tensor_scalar_add(q_sb[t][:, ssl], qp[:], pbq_t[:, t : t + 1])

        def emit_ln(t):
            nc.scalar.activation(kk_sb[t][:], kst[t][:], AF.Ln, bias=1.0, scale=1.0)

        def emit_vaug_ones(t):
            for rr in range(2):
                vv = vaug[2 * t + rr][:].rearrange("p (c w) -> p c w", w=65)
                nc.vector.tensor_copy(
                    vv[:, :, 64:65], ones_t.rearrange("p (c w) -> p c w", w=1)
                )

        def emit_vaug_chunk(t, j):
            """PE-transpose one [64,128] q chunk per head of half t."""
            for rr in range(2):
                lh = 2 * t + rr
                hsl = slice(rr * 64, rr * 64 + 64)
                tpv = ps.tile([128, 64], mmdt, tag="bp", name="tpv", bufs=1)
                nc.tensor.transpose(
                    tpv[:], q_sb[t][hsl, j * 128 : (j + 1) * 128], idb_t[hsl, 0:64]
                )
                nc.vector.tensor_copy(vaug[lh][:, j * 65 : j * 65 + 64], tpv[:])

        def emit_drain_chunk(prev_state, kc_rev, immediate=False):
            qc_p, t_p, ets_p, ctxs_p = prev_state
            for rr in range(2):
                nc.tensor.matmul(
                    ctxs_p[rr][:],
                    lhsT=vaug[2 * t_p + rr][:, kc_rev * 65 : kc_rev * 65 + 65],
                    rhs=ets_p[kc_rev][:, rr * 512 : rr * 512 + 512],
                    start=(kc_rev == (0 if immediate else NKC - 1)),
                    stop=(kc_rev == (NKC - 1 if immediate else 0)),
                )

        def emit_finalize(prev_state):
            qc_p, t_p, ets_p, ctxs_p = prev_state
            qsl_p = slice(qc_p * 512, (qc_p + 1) * 512)
            for rr in range(2):
                lh = 2 * t_p + rr
                cs = csp.tile([128, 512], f32, tag="cs")
                nc.vector.tensor_copy(cs[0:65, :], ctxs_p[rr][:])
                nc.sync.dma_start(out[lh * 64 : lh * 64 + 64, qsl_p], cs[0:64, :])
                nc.sync.dma_start(
                    dens[qc_p * 4 + t_p * 2 + rr : qc_p * 4 + t_p * 2 + rr + 1, :],
                    cs[64:65, :],
                )

        def run_filler(item):
            if item[0] == "vaug":
                emit_vaug_chunk(item[1], item[2])
            elif item[0] == "proj":
                emit_proj_chunk(item[1], item[2])
            elif item[0] == "ln":
                emit_ln(item[1])

        # ---------- schedule ----------
        # First-half projection, then one long pipeline of 8 t-major streams.
        for sc in range(NSC):
            emit_proj_chunk(0, sc)
        emit_ln(0)
        emit_vaug_ones(0)
        emit_vaug_ones(1)

        # extra PE work interleaved into the streams' spare PE slots
        filler = {
            0: [("vaug", 0, j) for j in range(NKC)],
            1: [("proj", 1, 0), ("proj", 1, 1)],
            2: [("proj", 1, 2), ("proj", 1, 3), ("ln", 1)],
            4: [("vaug", 1, j) for j in range(NKC)],
        }

        streams = [(qc, t) for t in range(2) for qc in range(NSC)]
        prev = None
        for i, (qc, t) in enumerate(streams):
            qsl = slice(qc * 512, (qc + 1) * 512)
            fill = list(filler.get(i, []))
            last = i == len(streams) - 1
            ets = []
            ctxs_now = None
            if last:
                # final stream: drain immediately per chunk (ascending kc), so
                # only the finalize remains after the pipeline.
                ctxs_now = [
                    ps.tile([65, 512], f32, tag="qp", name="ctxA", bufs=2),
                    ps.tile([65, 512], f32, tag="ap", name="ctxB", bufs=1),
                ]
            for kc in range(NKC):
                ksl = slice(kc * 128, (kc + 1) * 128)
                sp = ps.tile([128, 1024], f32, tag="sp", name="sp", bufs=2)
                nc.tensor.matmul(
                    sp[:, 0:512],
                    lhsT=kk_sb[t][0:64, ksl],
                    rhs=q_sb[t][0:64, qsl],
                    start=True,
                    stop=True,
                )
                nc.tensor.matmul(
                    sp[:, 512:1024],
                    lhsT=kk_sb[t][64:128, ksl],
                    rhs=q_sb[t][64:128, qsl],
                    start=True,
                    stop=True,
                )
                et = etp.tile([128, 1024], mmdt, tag="et", name=f"et{kc}")
                nc.scalar.activation(et[:], sp[:], AF.Exp, scale=0.125)
                ets.append(et)
                if prev is not None:
                    emit_drain_chunk(prev, NKC - 1 - kc)
                if last:
                    emit_drain_chunk((qc, t, ets, ctxs_now), kc, immediate=True)
                # interleave one filler item per chunk slot (back-loaded so the
                # filler's dependencies have time to resolve)
                if fill and (kc % 2 == 1 or len(fill) >= NKC - kc):
                    run_filler(fill.pop(0))
            for item in fill:
                run_filler(item)
            if prev is not None:
                emit_finalize(prev)
            if last:
                emit_finalize((qc, t, ets, ctxs_now))
                prev = None
            else:
                ctxs = [
                    ps.tile([65, 512], f32, tag="qp", name="ctxA", bufs=2),
                    ps.tile([65, 512], f32, tag="ap", name="ctxB", bufs=1),
                ]
                prev = (qc, t, ets, ctxs)

    nc.compile()
    return nc


def kernel(hidden_states, attention_mask, Wq, bq, Wk, bk):
    global _compiled, LAST_RESULT
    hs = np.asarray(hidden_states, dtype=np.float32)
    am = np.asarray(attention_mask)
    Wq = np.asarray(Wq, dtype=np.float32)
    Wk = np.asarray(Wk, dtype=np.float32)
    bq = np.asarray(bq, dtype=np.float32)
    bk = np.asarray(bk, dtype=np.float32)

    if _compiled is None:
        _compiled = _build()
    nc = _compiled

    from concourse.bass_utils import run_bass_kernel_spmd

    if MM_DTYPE == "bf16":
        import ml_dtypes

        def to_mmdt(x):
            return np.ascontiguousarray(
                np.asarray(x, np.float32).astype(ml_dtypes.bfloat16)
            )

    elif MM_DTYPE == "f32r":

        def to_mmdt(x):
            # fp32r = 1s/8e/11m (top 20 bits of fp32), round-to-nearest-even
            b = np.ascontiguousarray(x, dtype=np.float32).view(np.uint32)
            lsb = (b >> np.uint32(12)) & np.uint32(1)
            r = (b + np.uint32(0x7FF) + lsb) & np.uint32(0xFFFFF000)
            return r.view(np.float32)

    else:

        def to_mmdt(x):
            return np.ascontiguousarray(x, dtype=np.float32)

    idb = to_mmdt(np.tile(np.eye(64, dtype=np.float32), (2, 1)))
    in_maps = []
    for c in range(NCORES):
        b, g = c // HG, c % HG
        cols = slice(g * CPG, (g + 1) * CPG)
        bq_s = bq[cols].reshape(2, 128).T
        bk_s = bk[cols].reshape(2, 128).T
        smalls = np.concatenate(
            [bq_s, -(bq_s + bk_s), -(2 * bq_s + bk_s), np.ones((128, 16), np.float32)],
            axis=1,
        ).astype(np.float32)
        in_maps.append(
            {
                "ht": to_mmdt(hs[b].T),
                "wq": to_mmdt(Wq[:, cols]),
                "wa": to_mmdt(Wq[:, cols] + Wk[:, cols]),
                "wb": to_mmdt(2.0 * Wq[:, cols] + Wk[:, cols]),
                "smalls": np.ascontiguousarray(smalls),
                "idb": idb,
            }
        )

    res = run_bass_kernel_spmd(nc, in_maps, list(range(NCORES)))
    LAST_RESULT = res

    outp = np.empty((B, S, H * DH), dtype=np.float32)
    for c in range(NCORES):
        b, g = c // HG, c % HG
        ctxT = res.results[c]["out"]  # [256, 2048] raw ctx sums (transposed)
        dn = res.results[c]["dens"]  # [16, 512]: row qc*4 + t*2 + rr
        den = np.empty((4, S), dtype=np.float32)
        for qc in range(NSC):
            for t in range(2):
                for rr in range(2):
                    den[t * 2 + rr, qc * 512 : (qc + 1) * 512] = dn[qc * 4 + t * 2 + rr]
        ctxT = ctxT.reshape(4, 64, S) / den[:, None, :]
        outp[b, :, g * CPG : (g + 1) * CPG] = ctxT.reshape(CPG, S).T

    # attention_mask==0 masks whole query rows -> uniform probs -> ctx row is
    # the mean of q over all key positions. Never triggers for all-ones masks.
    if (am == 0).any():
        for b in range(B):
            rows = np.nonzero(am[b] == 0)[0]
            if rows.size:
                q_full = hs[b] @ Wq + bq
                outp[b, rows, :] = q_full.mean(axis=0)
    return outp
